# revision 6
# baseline (speedup 1.0000x reference)
"""Residual VQ (CLAP quantizer) TRN2 Bass kernel.

Problem: 8-stage residual vector quantization of [16, 2048, 512] f32
embeddings against 8x[1024, 512] codebooks. Returns (quantized [B,T,D],
indices [B,T,8] int32, commit_loss scalar).

Strategy (per spec sharding hint): data-parallel over the flattened token
axis -- 8 NeuronCores x 4096 tokens. Codebooks replicated. Per core:

  - residual row tiles res[t] [128 tok, 512 d] f32 persistent in SBUF
  - per stage q:
      * PE transposes res tile chunks (as f32r) -> resT [128 d, 128 tok]
      * dists via fp32r matmuls: s[tok, c] = 2*res.cb - |cb|^2
        (bias row folded in as a k=1 matmul; maximizing s == minimizing dist)
      * DVE max (top-8) + max_index -> argmin index per token
        (first-occurrence tie-break, same as jnp.argmin)
      * indirect DMA gather-accumulate: res[t] += (-codebook[idx]) directly
        from HBM (residual update fused into the gather DMA)
      * commit loss via recurrence ||res_new||^2 = ||res||^2 - s_max:
        only per-stage sums of s_max leave the device
  - quantized = emb - res_final (device); loss assembled on host.
"""

import numpy as np

import concourse.bass as bass
import concourse.mybir as mybir
from concourse.tile import TileContext
from concourse.bass_utils import run_bass_kernel_spmd

F32 = mybir.dt.float32
F32R = mybir.dt.float32r
U32 = mybir.dt.uint32

N_CORES = 8
B, T, D = 16, 2048, 512
N_TOK = B * T                 # 32768
TOK_PER_CORE = N_TOK // N_CORES   # 4096
N_TILES = TOK_PER_CORE // 128     # 32
Q, C = 8, 1024
KCH = D // 128                # 4 contraction chunks


def _split_multi_waits(nc):
    """This walrus build allows only one sync-wait per instruction; hoist
    extras onto same-engine NoOps inserted right before."""
    n_split = 0
    for func in nc.m.functions:
        for blk in func.blocks:
            newlist = []
            for ins in blk.instructions:
                si = ins.sync_info
                waits = list(si.on_wait) if si is not None and si.on_wait else []
                if len(waits) > 1:
                    for w in waits[:-1]:
                        nop = mybir.InstNoOp(
                            name=f"{ins.name}-wsplit-{n_split}",
                            engine=ins.engine,
                            ins=[], outs=[],
                            sync_info=mybir.SyncInfo(on_wait=[w], on_update=[]),
                        )
                        newlist.append(nop)
                        n_split += 1
                    ins.sync_info = mybir.SyncInfo(
                        on_wait=[waits[-1]],
                        on_update=list(si.on_update) if si.on_update else [],
                    )
                newlist.append(ins)
            blk.instructions = newlist
    return nc


def _build():
    nc = bass.Bass(trn_type="TRN2")

    emb = nc.dram_tensor("emb", [N_TILES, 128, D], F32, kind="ExternalInput")
    cbt2 = nc.dram_tensor("cbt2", [Q, 2, KCH, 128, C], F32R,
                          kind="ExternalInput")
    nbias = nc.dram_tensor("nbias", [Q, 2, C], F32R, kind="ExternalInput")
    ones2 = nc.dram_tensor("ones2", [2, 128], F32R, kind="ExternalInput")
    cbneg = nc.dram_tensor("cbneg", [Q * C, D], F32, kind="ExternalInput")
    ident = nc.dram_tensor("ident", [128, 128], F32, kind="ExternalInput")

    quant_o = nc.dram_tensor("quant_o", [N_TILES, 128, D], F32,
                             kind="ExternalOutput")
    idx_o = nc.dram_tensor("idx_o", [Q, 128, N_TILES], U32,
                           kind="ExternalOutput")
    msum_o = nc.dram_tensor("msum_o", [128, Q], F32, kind="ExternalOutput")

    with TileContext(nc) as tc:
        with (
            tc.tile_pool(name="res", bufs=1) as res_pool,
            tc.tile_pool(name="cb", bufs=2) as cb_pool,
            tc.tile_pool(name="work", bufs=3) as work_pool,
            tc.tile_pool(name="stage", bufs=2) as stage_pool,
            tc.tile_pool(name="fin", bufs=3) as fin_pool,
            tc.tile_pool(name="misc", bufs=1) as misc_pool,
            tc.tile_pool(name="tp_ps", bufs=2, space="PSUM") as tp_pool,
            tc.tile_pool(name="dist_ps", bufs=3, space="PSUM") as dist_pool,
        ):
            ident_t = misc_pool.tile([128, 128], F32, name="ident_t")
            nc.sync.dma_start(ident_t[:], ident[:])
            msum_t = misc_pool.tile([128, Q], F32, name="msum_t")
            ones2_t = misc_pool.tile([2, 128], F32R, name="ones2_t")
            nc.sync.dma_start(ones2_t[:], ones2[:])

            res_tiles = []
            for t in range(N_TILES):
                rt = res_pool.tile([128, D], F32, name=f"res{t}", tag=f"res{t}")
                nc.sync.dma_start(rt[:], emb[t])
                res_tiles.append(rt)

            for q in range(Q):
                cbt2_t = cb_pool.tile([128, 2, KCH, C], F32R, name="cbt2_t",
                                      tag="cbt2")
                nc.sync.dma_start(
                    cbt2_t[:], cbt2[q].rearrange("s k p c -> p s k c"))
                nbias_t = cb_pool.tile([2, C], F32R, name="nbias_t", tag="nbias")
                nc.sync.dma_start(nbias_t[:], nbias[q])

                m_all = stage_pool.tile([128, N_TILES], F32, name="m_all",
                                        tag="m_all")
                stg_all = stage_pool.tile([128, N_TILES], U32, name="stg_all",
                                          tag="stg_all")

                for t in range(N_TILES):
                    rt = res_tiles[t]
                    tp = tp_pool.tile([128, D], F32, name="tp")
                    for k in range(KCH):
                        nc.tensor.transpose(
                            tp[:, 128 * k:128 * (k + 1)],
                            rt[:, 128 * k:128 * (k + 1)],
                            ident_t[:],
                        )
                    resT = work_pool.tile([128, D], F32R, name="resT",
                                          tag="resT")
                    nc.scalar.copy(resT[:], tp[:])       # rounds to 12-bit hi
                    resL = work_pool.tile([128, D], F32R, name="resL",
                                          tag="resL")
                    nc.vector.tensor_tensor(
                        out=resL[:], in0=tp[:], in1=resT[:],
                        op=mybir.AluOpType.subtract)     # exact 12-bit lo

                    dp = dist_pool.tile([128, C], F32, name="dp")
                    # exact bias: k=2 matmul of the Dekker-split (-cbsq)
                    for h in range(2):
                        nc.tensor.matmul(
                            dp[:, 512 * h:512 * (h + 1)],
                            ones2_t[:],
                            nbias_t[:, 512 * h:512 * (h + 1)],
                            start=True, stop=False,
                        )
                    # Dekker 3-pass: (r12,c_hi), (rL,c_hi), (r12,c_lo)
                    mm_pairs = [(resT, 0), (resL, 0), (resT, 1)]
                    for pi, (lhs, s) in enumerate(mm_pairs):
                        last = pi == len(mm_pairs) - 1
                        for k in range(KCH):
                            for h in range(2):
                                nc.tensor.matmul(
                                    dp[:, 512 * h:512 * (h + 1)],
                                    lhs[:, 128 * k:128 * (k + 1)],
                                    cbt2_t[:, s, k, 512 * h:512 * (h + 1)],
                                    start=False,
                                    stop=(last and k == KCH - 1),
                                )

                    max8 = work_pool.tile([128, 8], F32, name="max8",
                                          tag="max8")
                    nc.vector.max(max8[:], dp[:])
                    idx8 = work_pool.tile([128, 8], U32, name="idx8",
                                          tag="idx8")
                    nc.vector.max_index(idx8[:], max8[:], dp[:])

                    nc.vector.tensor_copy(stg_all[:, t:t + 1], idx8[:, 0:1])
                    nc.vector.tensor_copy(m_all[:, t:t + 1], max8[:, 0:1])

                    # fused residual update: res += (-codebook[q][idx])
                    nc.gpsimd.indirect_dma_start(
                        out=rt[:],
                        out_offset=None,
                        in_=cbneg[:],
                        in_offset=bass.IndirectOffsetOnAxis(
                            ap=stg_all[:, t:t + 1], axis=0),
                        compute_op=mybir.AluOpType.add,
                        element_offset=q * C * D,
                    )

                nc.sync.dma_start(idx_o[q], stg_all[:])
                nc.vector.tensor_reduce(
                    msum_t[:, q:q + 1], m_all[:], axis=mybir.AxisListType.X,
                    op=mybir.AluOpType.add,
                )

            nc.sync.dma_start(msum_o[:], msum_t[:])

            for t in range(N_TILES):
                et = fin_pool.tile([128, D], F32, name="et", tag="et")
                nc.sync.dma_start(et[:], emb[t])
                qt = fin_pool.tile([128, D], F32, name="qt", tag="qt")
                nc.vector.tensor_tensor(
                    out=qt[:], in0=et[:], in1=res_tiles[t][:],
                    op=mybir.AluOpType.subtract,
                )
                nc.sync.dma_start(quant_o[t], qt[:])

    return _split_multi_waits(nc)


_NC_CACHE = None


def _get_nc():
    global _NC_CACHE
    if _NC_CACHE is None:
        _NC_CACHE = _build()
    return _NC_CACHE


def kernel(clap_embeddings: np.ndarray, codebooks: np.ndarray):
    clap_embeddings = np.ascontiguousarray(clap_embeddings, dtype=np.float32)
    codebooks = np.ascontiguousarray(codebooks, dtype=np.float32)

    flat = clap_embeddings.reshape(N_TOK, D)

    # host-prepped shared operands
    def split12(x):
        m_, e_ = np.frexp(x)
        hi = np.ldexp(np.round(m_ * 4096.0) / 4096.0, e_).astype(np.float32)
        lo = (x - hi).astype(np.float32)
        return hi, lo

    cbt2 = np.empty((Q, 2, KCH, 128, C), dtype=np.float32)
    for q in range(Q):
        cbt = codebooks[q].T  # [D, C]
        for k in range(KCH):
            chunk = (2.0 * cbt[128 * k:128 * (k + 1), :]).astype(np.float32)
            hi, lo = split12(chunk)
            cbt2[q, 0, k] = hi
            cbt2[q, 1, k] = lo
    cbsq = np.sum(
        codebooks * codebooks, axis=2, dtype=np.float32)  # [Q, C] f32
    ncbsq = -cbsq.astype(np.float32)
    # Dekker split into 12-bit-mantissa-safe hi + lo (fp32r rounds operands
    # to 12-bit mantissa; hi/lo pass through the PE rounding unchanged)
    bh, bl = split12(ncbsq)
    nbias = np.stack([bh, bl], axis=1)  # [Q, 2, C]
    ones2 = np.ones((2, 128), dtype=np.float32)
    cbneg = (-codebooks.reshape(Q * C, D)).astype(np.float32)
    ident = np.eye(128, dtype=np.float32)

    in_maps = []
    for c in range(N_CORES):
        shard = flat[c * TOK_PER_CORE:(c + 1) * TOK_PER_CORE]
        in_maps.append({
            "emb": np.ascontiguousarray(
                shard.reshape(N_TILES, 128, D)),
            "cbt2": cbt2,
            "nbias": nbias,
            "ones2": ones2,
            "cbneg": cbneg,
            "ident": ident,
        })

    nc = _get_nc()
    results = run_bass_kernel_spmd(
        nc, in_maps, core_ids=list(range(N_CORES))).results

    quant = np.empty((N_TOK, D), dtype=np.float32)
    indices = np.empty((N_TOK, Q), dtype=np.int32)
    s_sum = np.zeros(Q, dtype=np.float64)
    for c, r in enumerate(results):
        sl = slice(c * TOK_PER_CORE, (c + 1) * TOK_PER_CORE)
        quant[sl] = r["quant_o"].reshape(TOK_PER_CORE, D)
        # idx_o[q, p, t] -> token n = 128*t + p
        indices[sl] = r["idx_o"].transpose(2, 1, 0).reshape(
            TOK_PER_CORE, Q).astype(np.int32)
        s_sum += r["msum_o"].astype(np.float64).sum(axis=0)

    # commit loss: ssq_j = ssq0 - sum_{i<=j} S_i;  loss = sum_j ssq_j / (N*D)
    ssq0 = float(np.sum(flat.astype(np.float64) ** 2))
    coeff = np.array([Q - q for q in range(Q)], dtype=np.float64)
    loss = (Q * ssq0 - float(np.dot(coeff, s_sum))) / float(N_TOK * D)

    return (
        quant.reshape(B, T, D),
        indices.reshape(B, T, Q),
        np.float32(loss),
    )


# revision 9
# speedup vs baseline: 1.0013x; 1.0013x over previous
"""Residual VQ (CLAP quantizer) TRN2 Bass kernel.

Problem: 8-stage residual vector quantization of [16, 2048, 512] f32
embeddings against 8x[1024, 512] codebooks. Returns (quantized [B,T,D],
indices [B,T,8] int32, commit_loss scalar).

Strategy (per spec sharding hint): data-parallel over the flattened token
axis -- 8 NeuronCores x 4096 tokens. Codebooks replicated. Per core:

  - residual row tiles res[t] [128 tok, 512 d] f32 persistent in SBUF
  - per stage q:
      * PE transposes res tile chunks (f32, exact) -> tp psum; ACT copy
        rounds to the fp32r 12-bit-hi part, DVE subtract yields the exact
        12-bit lo part (Dekker split; fp32r HW rounds operands to 12-bit
        mantissa, products are then exact)
      * dists s[tok,c] = 2*res.cb - |cb|^2 via fp32r matmuls in 3 Dekker
        passes (hi*c_hi, hi*c_lo, lo*c_hi; lo*lo dropped ~2^-24) plus a
        k=2 matmul of the hi/lo-split bias -- full fp32-quality dists;
        maximizing s == minimizing distance
      * DVE max (top-8) + max_index -> argmin index per token
        (first-occurrence tie-break, same as jnp.argmin)
      * indirect DMA gather-accumulate: res[t] += (-codebook[idx]) directly
        from HBM (residual update fused into the gather DMA)
      * commit loss via recurrence ||res_new||^2 = ||res||^2 - s_max:
        only per-stage sums of s_max leave the device
  - quantized = emb - res_final (device); loss assembled on host.
"""

import numpy as np

import concourse.bass as bass
import concourse.mybir as mybir
from concourse.tile import TileContext
from concourse.bass_utils import run_bass_kernel_spmd

F32 = mybir.dt.float32
F32R = mybir.dt.float32r
U32 = mybir.dt.uint32

N_CORES = 8
B, T, D = 16, 2048, 512
N_TOK = B * T                 # 32768
TOK_PER_CORE = N_TOK // N_CORES   # 4096
N_TILES = TOK_PER_CORE // 128     # 32
Q, C = 8, 1024
KCH = D // 128                # 4 contraction chunks


def _split_multi_waits(nc):
    """This walrus build allows only one sync-wait per instruction; hoist
    extras onto same-engine NoOps inserted right before."""
    n_split = 0
    for func in nc.m.functions:
        for blk in func.blocks:
            newlist = []
            for ins in blk.instructions:
                si = ins.sync_info
                waits = list(si.on_wait) if si is not None and si.on_wait else []
                if len(waits) > 1:
                    for w in waits[:-1]:
                        nop = mybir.InstNoOp(
                            name=f"{ins.name}-wsplit-{n_split}",
                            engine=ins.engine,
                            ins=[], outs=[],
                            sync_info=mybir.SyncInfo(on_wait=[w], on_update=[]),
                        )
                        newlist.append(nop)
                        n_split += 1
                    ins.sync_info = mybir.SyncInfo(
                        on_wait=[waits[-1]],
                        on_update=list(si.on_update) if si.on_update else [],
                    )
                newlist.append(ins)
            blk.instructions = newlist
    return nc


def _build():
    nc = bass.Bass(trn_type="TRN2")

    emb = nc.dram_tensor("emb", [N_TILES, 128, D], F32, kind="ExternalInput")
    cbt2 = nc.dram_tensor("cbt2", [Q, 2, KCH, 128, C], F32R,
                          kind="ExternalInput")
    nbias = nc.dram_tensor("nbias", [Q, 2, C], F32R, kind="ExternalInput")
    ones2 = nc.dram_tensor("ones2", [2, 128], F32R, kind="ExternalInput")
    cbneg = nc.dram_tensor("cbneg", [Q * C, D], F32, kind="ExternalInput")
    ident = nc.dram_tensor("ident", [128, 128], F32, kind="ExternalInput")

    quant_o = nc.dram_tensor("quant_o", [N_TILES, 128, D], F32,
                             kind="ExternalOutput")
    idx_o = nc.dram_tensor("idx_o", [Q, 128, N_TILES], U32,
                           kind="ExternalOutput")
    msum_o = nc.dram_tensor("msum_o", [128, Q], F32, kind="ExternalOutput")

    with TileContext(nc) as tc:
        with (
            tc.tile_pool(name="res", bufs=1) as res_pool,
            tc.tile_pool(name="cb", bufs=2) as cb_pool,
            tc.tile_pool(name="work", bufs=6) as work_pool,
            tc.tile_pool(name="stage", bufs=3) as stage_pool,
            tc.tile_pool(name="fin", bufs=3) as fin_pool,
            tc.tile_pool(name="misc", bufs=1) as misc_pool,
            tc.tile_pool(name="tp_ps", bufs=2, space="PSUM") as tp_pool,
            tc.tile_pool(name="dist_ps", bufs=3, space="PSUM") as dist_pool,
        ):
            ident_t = misc_pool.tile([128, 128], F32, name="ident_t")
            nc.sync.dma_start(ident_t[:], ident[:])
            msum_t = misc_pool.tile([128, Q], F32, name="msum_t")
            ones2_t = misc_pool.tile([2, 128], F32R, name="ones2_t")
            nc.sync.dma_start(ones2_t[:], ones2[:])

            res_tiles = []
            for t in range(N_TILES):
                rt = res_pool.tile([128, D], F32, name=f"res{t}", tag=f"res{t}")
                nc.sync.dma_start(rt[:], emb[t])
                res_tiles.append(rt)

            for q in range(Q):
                cbt2_t = cb_pool.tile([128, 2, KCH, C], F32R, name="cbt2_t",
                                      tag="cbt2")
                nc.sync.dma_start(
                    cbt2_t[:], cbt2[q].rearrange("s k p c -> p s k c"))
                nbias_t = cb_pool.tile([2, C], F32R, name="nbias_t", tag="nbias")
                nc.sync.dma_start(nbias_t[:], nbias[q])

                m_all = stage_pool.tile([128, N_TILES], F32, name="m_all",
                                        tag="m_all")
                stg_all = stage_pool.tile([128, N_TILES], U32, name="stg_all",
                                          tag="stg_all")

                for t in range(N_TILES):
                    rt = res_tiles[t]
                    tp = tp_pool.tile([128, D], F32, name="tp")
                    for k in range(KCH):
                        nc.tensor.transpose(
                            tp[:, 128 * k:128 * (k + 1)],
                            rt[:, 128 * k:128 * (k + 1)],
                            ident_t[:],
                        )
                    resT = work_pool.tile([128, D], F32R, name="resT",
                                          tag="resT")
                    nc.scalar.copy(resT[:], tp[:])       # rounds to 12-bit hi
                    resL = work_pool.tile([128, D], F32R, name="resL",
                                          tag="resL")
                    nc.vector.tensor_tensor(
                        out=resL[:], in0=tp[:], in1=resT[:],
                        op=mybir.AluOpType.subtract)     # exact 12-bit lo

                    dp = dist_pool.tile([128, C], F32, name="dp")
                    # exact bias: k=2 matmul of the Dekker-split (-cbsq)
                    for h in range(2):
                        nc.tensor.matmul(
                            dp[:, 512 * h:512 * (h + 1)],
                            ones2_t[:],
                            nbias_t[:, 512 * h:512 * (h + 1)],
                            start=True, stop=False,
                        )
                    # Dekker 3-pass: (r12,c_hi), (r12,c_lo), (rL,c_hi).
                    # Same-lhsT matmuls are adjacent so each resT chunk's
                    # weights load once per 4 streams (LDW pull-ahead).
                    for k in range(KCH):
                        for s in range(2):
                            for h in range(2):
                                nc.tensor.matmul(
                                    dp[:, 512 * h:512 * (h + 1)],
                                    resT[:, 128 * k:128 * (k + 1)],
                                    cbt2_t[:, s, k, 512 * h:512 * (h + 1)],
                                    start=False, stop=False,
                                )
                    for k in range(KCH):
                        for h in range(2):
                            nc.tensor.matmul(
                                dp[:, 512 * h:512 * (h + 1)],
                                resL[:, 128 * k:128 * (k + 1)],
                                cbt2_t[:, 0, k, 512 * h:512 * (h + 1)],
                                start=False,
                                stop=(k == KCH - 1),
                            )

                    max8 = work_pool.tile([128, 8], F32, name="max8",
                                          tag="max8")
                    nc.vector.max(max8[:], dp[:])
                    idx8 = work_pool.tile([128, 8], U32, name="idx8",
                                          tag="idx8")
                    nc.vector.max_index(idx8[:], max8[:], dp[:])

                    nc.vector.tensor_copy(stg_all[:, t:t + 1], idx8[:, 0:1])
                    nc.vector.tensor_copy(m_all[:, t:t + 1], max8[:, 0:1])

                    # fused residual update: res += (-codebook[q][idx])
                    nc.gpsimd.indirect_dma_start(
                        out=rt[:],
                        out_offset=None,
                        in_=cbneg[:],
                        in_offset=bass.IndirectOffsetOnAxis(
                            ap=stg_all[:, t:t + 1], axis=0),
                        compute_op=mybir.AluOpType.add,
                        element_offset=q * C * D,
                    )

                nc.sync.dma_start(idx_o[q], stg_all[:])
                nc.vector.tensor_reduce(
                    msum_t[:, q:q + 1], m_all[:], axis=mybir.AxisListType.X,
                    op=mybir.AluOpType.add,
                )

            nc.sync.dma_start(msum_o[:], msum_t[:])

            for t in range(N_TILES):
                et = fin_pool.tile([128, D], F32, name="et", tag="et")
                nc.sync.dma_start(et[:], emb[t])
                qt = fin_pool.tile([128, D], F32, name="qt", tag="qt")
                nc.vector.tensor_tensor(
                    out=qt[:], in0=et[:], in1=res_tiles[t][:],
                    op=mybir.AluOpType.subtract,
                )
                nc.sync.dma_start(quant_o[t], qt[:])

    return _split_multi_waits(nc)


_NC_CACHE = None


def _get_nc():
    global _NC_CACHE
    if _NC_CACHE is None:
        _NC_CACHE = _build()
    return _NC_CACHE


def kernel(clap_embeddings: np.ndarray, codebooks: np.ndarray):
    clap_embeddings = np.ascontiguousarray(clap_embeddings, dtype=np.float32)
    codebooks = np.ascontiguousarray(codebooks, dtype=np.float32)

    flat = clap_embeddings.reshape(N_TOK, D)

    # host-prepped shared operands
    def split12(x):
        m_, e_ = np.frexp(x)
        hi = np.ldexp(np.round(m_ * 4096.0) / 4096.0, e_).astype(np.float32)
        lo = (x - hi).astype(np.float32)
        return hi, lo

    cbt2 = np.empty((Q, 2, KCH, 128, C), dtype=np.float32)
    for q in range(Q):
        cbt = codebooks[q].T  # [D, C]
        for k in range(KCH):
            chunk = (2.0 * cbt[128 * k:128 * (k + 1), :]).astype(np.float32)
            hi, lo = split12(chunk)
            cbt2[q, 0, k] = hi
            cbt2[q, 1, k] = lo
    cbsq = np.sum(
        codebooks * codebooks, axis=2, dtype=np.float32)  # [Q, C] f32
    ncbsq = -cbsq.astype(np.float32)
    # Dekker split into 12-bit-mantissa-safe hi + lo (fp32r rounds operands
    # to 12-bit mantissa; hi/lo pass through the PE rounding unchanged)
    bh, bl = split12(ncbsq)
    nbias = np.stack([bh, bl], axis=1)  # [Q, 2, C]
    ones2 = np.ones((2, 128), dtype=np.float32)
    cbneg = (-codebooks.reshape(Q * C, D)).astype(np.float32)
    ident = np.eye(128, dtype=np.float32)

    in_maps = []
    for c in range(N_CORES):
        shard = flat[c * TOK_PER_CORE:(c + 1) * TOK_PER_CORE]
        in_maps.append({
            "emb": np.ascontiguousarray(
                shard.reshape(N_TILES, 128, D)),
            "cbt2": cbt2,
            "nbias": nbias,
            "ones2": ones2,
            "cbneg": cbneg,
            "ident": ident,
        })

    nc = _get_nc()
    results = run_bass_kernel_spmd(
        nc, in_maps, core_ids=list(range(N_CORES))).results

    quant = np.empty((N_TOK, D), dtype=np.float32)
    indices = np.empty((N_TOK, Q), dtype=np.int32)
    s_sum = np.zeros(Q, dtype=np.float64)
    for c, r in enumerate(results):
        sl = slice(c * TOK_PER_CORE, (c + 1) * TOK_PER_CORE)
        quant[sl] = r["quant_o"].reshape(TOK_PER_CORE, D)
        # idx_o[q, p, t] -> token n = 128*t + p
        indices[sl] = r["idx_o"].transpose(2, 1, 0).reshape(
            TOK_PER_CORE, Q).astype(np.int32)
        s_sum += r["msum_o"].astype(np.float64).sum(axis=0)

    # commit loss: ssq_j = ssq0 - sum_{i<=j} S_i;  loss = sum_j ssq_j / (N*D)
    ssq0 = float(np.sum(flat.astype(np.float64) ** 2))
    coeff = np.array([Q - q for q in range(Q)], dtype=np.float64)
    loss = (Q * ssq0 - float(np.dot(coeff, s_sum))) / float(N_TOK * D)

    return (
        quant.reshape(B, T, D),
        indices.reshape(B, T, Q),
        np.float32(loss),
    )
